# revision 33
# baseline (speedup 1.0000x reference)
"""DGCNN (nn_DGCNN_param_57904749085240) Trainium2 Bass kernel.

Data-parallel over batch: 8 cores x 2 point clouds each, no collectives.

Per EdgeConv layer, instead of materializing (2C, N, k) edge features, use
    W @ [x_j - x_i; x_i] = W1 x_j + (W2 - W1) x_i
and eval-BN + leaky folding (per-channel scale s > 0 commutes with max_k):
    y[:, i] = leaky( max_{j in knn(i)} (A x_j)  +  Cc x_i + t )
with A = s*W1, Cc = s*(W2-W1), t = s*b + beta - s*mu, all host-folded.

knn is exact-ish fp32r: pd = 2 X^T X - xx_i - xx_j comes out of the PE via an
augmented matmul ([X | ones | xx] pairs with [2X | -xx | -ones]); per
128-point tile the top-20 indices are 3 rounds of DVE max8/max_index/
match_replace.  Neighbor max = gpsimd ap_gather (SBUF->SBUF, fp32) with a
wrapped 16-partition index list + grouped DVE tensor_reduce max.

Perf structure vs the fp32 baseline: wide matmuls run as float32r
(1 cycle/row vs 4), all leaky-relus ride the Activation engine (AF.Lrelu)
fused with the PSUM evacuation, and the big lc/l0/l1 weights are resident
in SBUF instead of streamed per-use.
"""
import sys

sys.path.insert(0, "/opt/trn_rl_repo")

import numpy as np

import concourse.bacc as bacc
import concourse.tile as tile
from concourse import mybir
from concourse.bass_utils import run_bass_kernel_spmd

F32 = mybir.dt.float32
F32R = mybir.dt.float32r
I16 = mybir.dt.int16
U16 = mybir.dt.uint16

B, N, K = 16, 1024, 20
N_CORES = 8
ELS = B // N_CORES
CH_C = [3, 64, 64, 128]
CH_O = [64, 64, 128, 256]
EMB = 1024
NT = N // 128
MMF = 512                     # fp32 matmul free-dim limit (one PSUM bank)
NEG = -1.0e30

AF = mybir.ActivationFunctionType
ALU = mybir.AluOpType
AX = mybir.AxisListType


def _mm(nc, out, lhsT, rhs, start, stop, rdt=True):
    if rdt:
        lhsT = lhsT.bitcast(F32R)
        rhs = rhs.bitcast(F32R)
    fd = rhs.shape[-1]
    if fd <= MMF:
        nc.tensor.matmul(out=out, lhsT=lhsT, rhs=rhs, start=start, stop=stop)
        return
    for f0 in range(0, fd, MMF):
        f1 = min(f0 + MMF, fd)
        nc.tensor.matmul(out=out[:, f0:f1], lhsT=lhsT, rhs=rhs[:, f0:f1],
                         start=start, stop=stop)


def _pool_kmax(nc, m_seg, g):
    """Neighbor max over K=20 on the Pool engine: in-place max tree inside
    the gathered tile g [128, 128*K], result -> m_seg [128, 128]."""
    gv = g.rearrange("p (i r) -> p i r", r=K)
    TT = nc.gpsimd.tensor_tensor
    TT(out=gv[:, :, 0:10], in0=gv[:, :, 0:10], in1=gv[:, :, 10:20], op=ALU.max)
    TT(out=gv[:, :, 0:5], in0=gv[:, :, 0:5], in1=gv[:, :, 5:10], op=ALU.max)
    TT(out=gv[:, :, 0:2], in0=gv[:, :, 0:2], in1=gv[:, :, 2:4], op=ALU.max)
    TT(out=gv[:, :, 0:1], in0=gv[:, :, 0:1], in1=gv[:, :, 1:2], op=ALU.max)
    TT(out=m_seg.rearrange("p (i r) -> p i r", r=1),
       in0=gv[:, :, 0:1], in1=gv[:, :, 4:5], op=ALU.max)


def build_program(debug=False, reps=1, ablate=()):
    nc = bacc.Bacc("TRN2", target_bir_lowering=False, debug=False)

    x_in = nc.dram_tensor("x3", [ELS * 3, N], F32, kind="ExternalInput")
    wa_d, wc_d, wt_d = [], [], []
    for l in range(4):
        C, O = CH_C[l], CH_O[l]
        wa_d.append(nc.dram_tensor(f"wa{l}", [C, O], F32, kind="ExternalInput"))
        wc_d.append(nc.dram_tensor(f"wc{l}", [C, O], F32, kind="ExternalInput"))
        wt_d.append(nc.dram_tensor(f"wt{l}", [1, O], F32, kind="ExternalInput"))
    wlc_d = nc.dram_tensor("wlc", [512, EMB], F32, kind="ExternalInput")
    lct_d = nc.dram_tensor("lct", [128, NT], F32, kind="ExternalInput")
    wl0_d = nc.dram_tensor("wl0", [2049, 512], F32, kind="ExternalInput")
    wl1_d = nc.dram_tensor("wl1", [513, 256], F32, kind="ExternalInput")
    wow_d = nc.dram_tensor("wow", [257, 40], F32, kind="ExternalInput")
    out_d = nc.dram_tensor("out", [ELS, 40], F32, kind="ExternalOutput")

    lc_rows = [(0, 64), (64, 128), (128, 256), (256, 384), (384, 512)]
    l0_rows = [(kc * 128, (kc + 1) * 128) for kc in range(16)] + [(2048, 2049)]
    l1_rows = [(kc * 128, (kc + 1) * 128) for kc in range(4)] + [(512, 513)]
    ow_rows = [(0, 128), (128, 256), (256, 257)]

    with tile.TileContext(nc) as tc:
        with (
            tc.tile_pool(name="w", bufs=1) as wpool,
            tc.tile_pool(name="y", bufs=1) as ypool,
            tc.tile_pool(name="s1", bufs=1) as spool1,
            tc.tile_pool(name="s", bufs=2) as spool,
            tc.tile_pool(name="pdp", bufs=3) as pdpool,
            tc.tile_pool(name="g", bufs=3) as gpool,
            tc.tile_pool(name="iwp", bufs=1) as iwpool,
            tc.tile_pool(name="dr", bufs=2, space="DRAM") as dramp,
            tc.tile_pool(name="jit", bufs=2) as jitp,
        ):
            # ---------------- consts + resident weights ----------------
            ones_row = wpool.tile([1, N], F32, tag="ones_row")
            ones_col = wpool.tile([128, 1], F32, tag="ones_col")
            ones2 = wpool.tile([1, ELS], F32, tag="ones2")
            eye2 = wpool.tile([ELS, ELS], F32, tag="eye2")
            nc.vector.memset(ones_row[:], 1.0)
            nc.vector.memset(ones_col[:], 1.0)
            nc.vector.memset(ones2[:], 1.0)
            nc.vector.memset(eye2[:], 0.0)
            for _e in range(ELS):
                nc.vector.memset(eye2[_e:_e + 1, _e:_e + 1], 1.0)

            x0_tiles = []
            for el in range(ELS):
                t = ypool.tile([3, N], F32, tag=f"x0_{el}", name=f"x0_{el}")
                nc.sync.dma_start(t[:], x_in.ap()[el * 3:(el + 1) * 3, :])
                x0_tiles.append(t)

            wa, wc, wt = [], [], []
            for l in range(4):
                C, O = CH_C[l], CH_O[l]
                ta = wpool.tile([C, O], F32, tag=f"wa{l}")
                tcc = wpool.tile([C, O], F32, tag=f"wc{l}")
                tt = wpool.tile([1, O], F32, tag=f"wt{l}")
                nc.sync.dma_start(ta[:], wa_d[l].ap())
                nc.sync.dma_start(tcc[:], wc_d[l].ap())
                nc.sync.dma_start(tt[:], wt_d[l].ap())
                wa.append(ta); wc.append(tcc); wt.append(tt)

            # resident lc weights + per-channel bias column, loaded once
            wlc_t = []
            for r0, r1 in lc_rows:
                t = wpool.tile([r1 - r0, EMB], F32, tag=f"wlc{r0}")
                nc.sync.dma_start(t[:], wlc_d.ap()[r0:r1, :])
                wlc_t.append(t)
            lct = wpool.tile([128, NT], F32, tag="lct")
            nc.sync.dma_start(lct[:], lct_d.ap())


            # h_parts[l][el] = list of ([<=128, N] AP) feature chunks (lc concat order)
            h_parts = [[None] * ELS for _ in range(4)]
            maxes = ypool.tile([128, NT, ELS], F32, tag="maxes")
            sums = ypool.tile([128, NT, ELS], F32, tag="sums")
            aug_t, raug_t = [], []
            for el in range(ELS):
                a1 = wpool.tile([2, N], F32, tag=f"aug{el}", name=f"aug{el}")
                a2 = wpool.tile([2, N], F32, tag=f"raug{el}", name=f"raug{el}")
                nc.sync.dma_start(a1[1:2, :], ones_row[:])
                nc.sync.dma_start(a2[0:1, :], ones_row[:])
                aug_t.append(a1); raug_t.append(a2)

            for _rep in range(reps):
              Xf = [x0_tiles[el][:] for el in range(ELS)]
              with (
                tc.tile_pool(name=f"pspd{_rep}", bufs=2, space="PSUM") as pspd,
                tc.tile_pool(name=f"psmm{_rep}", bufs=1, space="PSUM") as psmm,
                tc.tile_pool(name=f"pslc{_rep}", bufs=1, space="PSUM") as pslc,
              ):
                # ================= EdgeConv layers =================
                for l in range(4):
                    C, O = CH_C[l], CH_O[l]
                    packed = (O == 64 and ELS == 2)
                    nch = 1 if packed else O // 128  # gather-channel chunks per el

                    # per-tile wrapped+replicated idx tiles (separate tiles so a
                    # gather depends only on its own tile's topk, not all of them)
                    iw_t = {}
                    for t in range(NT):
                        if packed:
                            w = iwpool.tile([128, 160], I16, tag=f"iw{t}", name=f"iw{t}")
                            for el in range(ELS):
                                iw_t[(el, t)] = w
                        else:
                            for el in range(ELS):
                                w = iwpool.tile([128, 160], I16, tag=f"iw{el}_{t}",
                                                name=f"iw{el}_{t}")
                                iw_t[(el, t)] = w

                    # per-el prep: xx row + aug rows (pd/2 variant: the 2x
                    # rhs scaling is dropped; ordering is unchanged)
                    flats = []
                    for el in range(ELS):
                        xf = Xf[el]
                        xsq = spool1.tile([C, N], F32, tag=f"xsq{el}")
                        nc.scalar.activation(out=xsq[:], in_=xf, func=AF.Square)
                        xx_ps = psmm.tile([1, N], F32, tag="mm")
                        _mm(nc, xx_ps[:], ones_col[0:C, :], xsq[:], True, True)
                        # pd/2 = G + nxx_i/2 + nxx_j/2: aug = [nxx/2; 1]
                        aug, raug = aug_t[el], raug_t[el]
                        nc.scalar.activation(out=aug[0:1, :], in_=xx_ps[:], func=AF.Copy,
                                             scale=-0.5)
                        nc.sync.dma_start(raug[1:2, :], aug[0:1, :])
                        flat = dramp.tile([NT * 128, K], I16, tag=f"idxflat{el}",
                                          name=f"idxflat{el}")
                        flats.append(flat)

                    # topk per tile
                    for el in range(ELS):
                        for t in range(NT):
                            xf = Xf[el]
                            iw = iw_t[(el, t)]
                            p_base = 64 * el if packed else 0
                            nrep = 4 if packed else 8
                            aug, raug = aug_t[el], raug_t[el]
                            flat = flats[el]

                            pd_ps = pspd.tile([128, N], F32, tag="pd")
                            _mm(nc, pd_ps[:], xf[:, t * 128:(t + 1) * 128], xf,
                                True, False)
                            _mm(nc, pd_ps[:], aug[:, t * 128:(t + 1) * 128], raug[:],
                                False, True)
                            pd_sb = pdpool.tile([128, N], F32, tag="pdsb")
                            nc.scalar.activation(out=pd_sb[:], in_=pd_ps[:], func=AF.Copy)
                            v = pdpool.tile([128, 24], F32, tag="v")
                            vi = pdpool.tile([128, 24], U16, tag="vi")
                            nc.vector.max(out=v[:, 0:8], in_=pd_sb[:])
                            nc.vector.max_index(out=vi[:, 0:8], in_max=v[:, 0:8],
                                                in_values=pd_sb[:])
                            nc.vector.match_replace(out=pd_sb[:], in_to_replace=v[:, 0:8],
                                                    in_values=pd_sb[:], imm_value=NEG)
                            nc.vector.max(out=v[:, 8:16], in_=pd_sb[:])
                            nc.vector.max_index(out=vi[:, 8:16], in_max=v[:, 8:16],
                                                in_values=pd_sb[:])
                            nc.vector.match_replace(out=pd_sb[:], in_to_replace=v[:, 8:16],
                                                    in_values=pd_sb[:], imm_value=NEG)
                            nc.vector.max(out=v[:, 16:24], in_=pd_sb[:])
                            nc.vector.max_index(out=vi[:, 16:24], in_max=v[:, 16:24],
                                                in_values=pd_sb[:])
                            # e-order dump -> DRAM, then wrapped([16,160]) via
                            # per-tile: e-order dump -> wrapped strided read
                            # -> log2 replication (pipelines under rounds)
                            nc.sync.dma_start(flat[t * 128:(t + 1) * 128, :],
                                              vi[:, 0:K].bitcast(I16))
                            src = (flat[t * 128:(t + 1) * 128, :]
                                   .rearrange("p r -> (p r)")
                                   .rearrange("(s w) -> w s", w=16))
                            nc.sync.dma_start(iw[p_base:p_base + 16, :], src)
                            blk = 16
                            while blk < 16 * nrep:
                                nc.sync.dma_start(
                                    iw[p_base + blk:p_base + 2 * blk, :],
                                    iw[p_base:p_base + blk, :])
                                blk *= 2

                    # ---- convs + gather + activation ----
                    if packed:
                        a_sb = spool.tile([128, N], F32, tag="asb")
                        c_sb = spool.tile([128, N], F32, tag="csb")
                        for el in range(ELS):
                            a_ps = psmm.tile([64, N], F32, tag="mm")
                            _mm(nc, a_ps[:], wa[l][:, 0:O], Xf[el], True, True)
                            nc.scalar.activation(out=a_sb[64 * el:64 * (el + 1), :],
                                                 in_=a_ps[:], func=AF.Copy)
                            c_ps = psmm.tile([64, N], F32, tag="mm")
                            _mm(nc, c_ps[:], wc[l][:, 0:O], Xf[el], True, False)
                            _mm(nc, c_ps[:], wt[l][:, 0:O], ones_row[:], False, True)
                            nc.scalar.activation(out=c_sb[64 * el:64 * (el + 1), :],
                                                 in_=c_ps[:], func=AF.Copy)
                        m_sb = spool.tile([128, N], F32, tag="msb")
                        for t in range(NT):
                            g = gpool.tile([128, 2560], F32, tag="gath")
                            nc.gpsimd.ap_gather(
                                out_ap=g[:], in_ap=a_sb[:],
                                idxs_ap=iw_t[(0, t)][:],
                                channels=128, num_elems=N, d=1, num_idxs=2560)
                            _pool_kmax(nc, m_sb[:, t * 128:(t + 1) * 128], g[:])
                        u = spool.tile([128, N], F32, tag="u")
                        nc.gpsimd.tensor_tensor(out=u[:], in0=m_sb[:], in1=c_sb[:],
                                                op=ALU.add)
                        newX = []
                        for el in range(ELS):
                            yt = ypool.tile([64, N], F32, tag=f"y{l}_{el}",
                                            name=f"y{l}_{el}")
                            nc.scalar.activation(out=yt[:],
                                                 in_=u[64 * el:64 * (el + 1), :],
                                                 func=AF.Lrelu, alpha=0.2)
                            h_parts[l][el] = [yt[:]]
                            newX.append(yt[:])
                        Xf = newX
                    else:
                        newX = [None] * ELS
                        for el in range(ELS):
                            ychunks = []
                            for ch in range(nch):
                                o0, o1 = ch * 128, (ch + 1) * 128
                                a_sb = spool.tile([128, N], F32, tag="asb")
                                a_ps = psmm.tile([128, N], F32, tag="mm")
                                _mm(nc, a_ps[:], wa[l][:, o0:o1], Xf[el], True, True)
                                nc.scalar.activation(out=a_sb[:], in_=a_ps[:],
                                                     func=AF.Copy)
                                c_ps = psmm.tile([128, N], F32, tag="mm")
                                _mm(nc, c_ps[:], wc[l][:, o0:o1], Xf[el], True, False)
                                _mm(nc, c_ps[:], wt[l][:, o0:o1], ones_row[:],
                                    False, True)
                                c_sb = spool.tile([128, N], F32, tag="csb")
                                nc.scalar.activation(out=c_sb[:], in_=c_ps[:],
                                                     func=AF.Copy)
                                m_sb = spool.tile([128, N], F32, tag="msb")
                                for t in range(NT):
                                    g = gpool.tile([128, 2560], F32, tag="gath")
                                    nc.gpsimd.ap_gather(
                                        out_ap=g[:], in_ap=a_sb[:],
                                        idxs_ap=iw_t[(el, t)][:],
                                        channels=128, num_elems=N, d=1, num_idxs=2560)
                                    if l == 3 and t % 2 == 1:
                                        nc.vector.tensor_reduce(
                                            out=m_sb[:, t * 128:(t + 1) * 128],
                                            in_=g[:].rearrange("p (i r) -> p i r", r=K),
                                            axis=AX.X, op=ALU.max)
                                    else:
                                        _pool_kmax(nc, m_sb[:, t * 128:(t + 1) * 128],
                                                   g[:])
                                u = spool.tile([128, N], F32, tag="u")
                                nc.gpsimd.tensor_tensor(out=u[:], in0=m_sb[:],
                                                        in1=c_sb[:], op=ALU.add)
                                yt = ypool.tile([128, N], F32, tag=f"y{l}_{el}_{ch}")
                                nc.scalar.activation(out=yt[:], in_=u[:],
                                                     func=AF.Lrelu, alpha=0.2)
                                ychunks.append(yt[:])
                            h_parts[l][el] = ychunks
                            if nch == 1:
                                newX[el] = ychunks[0]
                        if l < 3:
                            Xf = newX

                    # ================= lc conv + pooling =================
                for el in range(ELS):
                    rhs_chunks = (h_parts[0][el] + h_parts[1][el] + h_parts[2][el]
                                  + h_parts[3][el])
                    for mt in range(8):
                        u_ps = pslc.tile([128, N], F32, tag="lc")
                        for kc in range(5):
                            _mm(nc, u_ps[:], wlc_t[kc][:, mt * 128:(mt + 1) * 128],
                                rhs_chunks[kc], kc == 0, kc == 4)
                        y5 = spool.tile([128, N], F32, tag="y5")
                        nc.scalar.activation(out=y5[:], in_=u_ps[:], func=AF.Lrelu,
                                             alpha=0.2, bias=lct[:, mt:mt + 1],
                                             accum_out=sums[:, mt:mt + 1, el:el + 1])
                        nc.vector.tensor_reduce(out=maxes[:, mt:mt + 1, el:el + 1],
                                                in_=y5[:], axis=AX.X, op=ALU.max)

              # ====== FC head: batch (ELS) on PSUM partitions, wide free dims ======
              with tc.tile_pool(name=f"psfc{_rep}", bufs=1, space="PSUM") as psfc:
                  # l0: out [ELS, 512] accumulated over 17 contract chunks
                  l0ps = psfc.tile([ELS, 512], F32, tag="fc0")
                  for kc in range(17):
                      r0, r1 = l0_rows[kc]
                      wj = jitp.tile([r1 - r0, 512], F32, tag="wj0")
                      nc.sync.dma_start(wj[:], wl0_d.ap()[r0:r1, :])
                      if kc < 8:
                          lhsT = maxes[:, kc:kc + 1, :].rearrange("p a b -> p (a b)")
                      elif kc < 16:
                          lhsT = sums[:, kc - 8:kc - 7, :].rearrange("p a b -> p (a b)")
                      else:
                          lhsT = ones2[:]
                      nc.tensor.matmul(out=l0ps[:], lhsT=lhsT, rhs=wj[:],
                                       start=kc == 0, stop=kc == 16)
                  y6 = spool.tile([ELS, 512], F32, tag="y6")
                  nc.scalar.activation(out=y6[:], in_=l0ps[:], func=AF.Lrelu,
                                       alpha=0.2)
                  # transpose y6 -> [128, 4, ELS] via 4 PE transposes
                  y6tp = psfc.tile([128, 4, ELS], F32, tag="fct")
                  for c in range(4):
                      nc.tensor.transpose(out=y6tp[:, c, :],
                                          in_=y6[:, c * 128:(c + 1) * 128],
                                          identity=eye2[:])
                  y6t = spool.tile([128, 4 * ELS], F32, tag="y6t")
                  y6tv = y6t[:].rearrange("p (a b) -> p a b", a=4)
                  nc.scalar.activation(out=y6t[:],
                                       in_=y6tp[:].rearrange("p a b -> p (a b)"),
                                       func=AF.Copy)
                  # l1: out [ELS, 256] over 4 contract chunks + bias row
                  l1ps = psfc.tile([ELS, 256], F32, tag="fc1")
                  for kc in range(5):
                      r0, r1 = l1_rows[kc]
                      wj = jitp.tile([r1 - r0, 256], F32, tag="wj1")
                      nc.sync.dma_start(wj[:], wl1_d.ap()[r0:r1, :])
                      lhsT = (y6tv[:, kc, :] if kc < 4 else ones2[:])
                      nc.tensor.matmul(out=l1ps[:], lhsT=lhsT, rhs=wj[:],
                                       start=kc == 0, stop=kc == 4)
                  y7 = spool.tile([ELS, 256], F32, tag="y7")
                  nc.scalar.activation(out=y7[:], in_=l1ps[:], func=AF.Lrelu,
                                       alpha=0.2)
                  y7tp = psfc.tile([128, 2, ELS], F32, tag="fct2")
                  for c in range(2):
                      nc.tensor.transpose(out=y7tp[:, c, :],
                                          in_=y7[:, c * 128:(c + 1) * 128],
                                          identity=eye2[:])
                  y7t = spool.tile([128, 2 * ELS], F32, tag="y7t")
                  y7tv = y7t[:].rearrange("p (a b) -> p a b", a=2)
                  nc.scalar.activation(out=y7t[:],
                                       in_=y7tp[:].rearrange("p a b -> p (a b)"),
                                       func=AF.Copy)
                  ops_ = psfc.tile([ELS, 40], F32, tag="fcout")
                  for kc in range(3):
                      r0, r1 = ow_rows[kc]
                      wj = jitp.tile([r1 - r0, 40], F32, tag=f"wjo{kc}")
                      nc.sync.dma_start(wj[:], wow_d.ap()[r0:r1, :])
                      lhsT = (y7tv[:, kc, :] if kc < 2 else ones2[:])
                      nc.tensor.matmul(out=ops_[:], lhsT=lhsT, rhs=wj[:],
                                       start=kc == 0, stop=kc == 2)
                  osb = spool.tile([ELS, 40], F32, tag="osb")
                  nc.scalar.activation(out=osb[:], in_=ops_[:], func=AF.Copy)
                  nc.sync.dma_start(out_d.ap(), osb[:])

    nc.compile()
    return nc


def _fold_weights(i):
    out = {}
    for l in range(4):
        C = CH_C[l]
        w = np.asarray(i[f"c{l}_w"], np.float64)
        b = np.asarray(i[f"c{l}_b"], np.float64)
        g = np.asarray(i[f"c{l}_g"], np.float64)
        be = np.asarray(i[f"c{l}_be"], np.float64)
        m = np.asarray(i[f"c{l}_m"], np.float64)
        v = np.asarray(i[f"c{l}_v"], np.float64)
        s = g / np.sqrt(v + 1e-5)
        w1, w2 = w[:, :C], w[:, C:]
        out[f"wa{l}"] = np.ascontiguousarray((s[:, None] * w1).T).astype(np.float32)
        out[f"wc{l}"] = np.ascontiguousarray((s[:, None] * (w2 - w1)).T).astype(np.float32)
        out[f"wt{l}"] = (s * b + be - s * m).astype(np.float32)[None, :]
    s = np.asarray(i["lc_g"], np.float64) / np.sqrt(np.asarray(i["lc_v"], np.float64) + 1e-5)
    t = s * np.asarray(i["lc_b"], np.float64) + np.asarray(i["lc_be"], np.float64) \
        - s * np.asarray(i["lc_m"], np.float64)
    out["wlc"] = np.ascontiguousarray(
        np.concatenate([(s[:, None] * np.asarray(i["lc_w"], np.float64)).T,
                        t[None, :]], 0)).astype(np.float32)
    s = np.asarray(i["l0_g"], np.float64) / np.sqrt(np.asarray(i["l0_v"], np.float64) + 1e-5)
    t = np.asarray(i["l0_be"], np.float64) - s * np.asarray(i["l0_m"], np.float64)
    w = s[:, None] * np.asarray(i["l0_w"], np.float64)
    w[:, 1024:] /= 1024.0
    out["wl0"] = np.ascontiguousarray(np.concatenate([w.T, t[None, :]], 0)).astype(np.float32)
    s = np.asarray(i["l1_g"], np.float64) / np.sqrt(np.asarray(i["l1_v"], np.float64) + 1e-5)
    t = s * np.asarray(i["l1_b"], np.float64) + np.asarray(i["l1_be"], np.float64) \
        - s * np.asarray(i["l1_m"], np.float64)
    out["wl1"] = np.ascontiguousarray(
        np.concatenate([(s[:, None] * np.asarray(i["l1_w"], np.float64)).T,
                        t[None, :]], 0)).astype(np.float32)
    out["wow"] = np.ascontiguousarray(
        np.concatenate([np.asarray(i["ow"], np.float32).T,
                        np.asarray(i["ob"], np.float32)[None, :]], 0))
    return out


_NC_CACHE = {}


def get_program(debug=False):
    if debug not in _NC_CACHE:
        _NC_CACHE[debug] = build_program(debug)
    return _NC_CACHE[debug]


def make_in_maps(inputs):
    folded = _fold_weights(inputs)
    x = np.asarray(inputs["x"], np.float32)
    in_maps = []
    for c in range(N_CORES):
        m = dict(folded)
        xs = x[c * ELS:(c + 1) * ELS]                       # (ELS, 1024, 3)
        m["x3"] = np.ascontiguousarray(
            xs.transpose(0, 2, 1).reshape(ELS * 3, N))
        in_maps.append(m)
    return in_maps


def kernel(**inputs) -> np.ndarray:
    nc = get_program(False)
    in_maps = make_in_maps(inputs)
    res = run_bass_kernel_spmd(nc, in_maps, list(range(N_CORES)))
    outs = [res.results[c]["out"] for c in range(N_CORES)]
    return np.concatenate(outs, 0).astype(np.float32)


# revision 35
# speedup vs baseline: 1.0217x; 1.0217x over previous
"""DGCNN (nn_DGCNN_param_57904749085240) Trainium2 Bass kernel.

Data-parallel over batch: 8 cores x 2 point clouds each, no collectives.

Per EdgeConv layer, instead of materializing (2C, N, k) edge features, use
    W @ [x_j - x_i; x_i] = W1 x_j + (W2 - W1) x_i
and eval-BN + leaky folding (per-channel scale s > 0 commutes with max_k):
    y[:, i] = leaky( max_{j in knn(i)} (A x_j)  +  Cc x_i + t )
with A = s*W1, Cc = s*(W2-W1), t = s*b + beta - s*mu, all host-folded.

knn is exact-ish fp32r: pd = 2 X^T X - xx_i - xx_j comes out of the PE via an
augmented matmul ([X | ones | xx] pairs with [2X | -xx | -ones]); per
128-point tile the top-20 indices are 3 rounds of DVE max8/max_index/
match_replace.  Neighbor max = gpsimd ap_gather (SBUF->SBUF, fp32) with a
wrapped 16-partition index list + grouped DVE tensor_reduce max.

Perf structure vs the fp32 baseline: wide matmuls run as float32r
(1 cycle/row vs 4), all leaky-relus ride the Activation engine (AF.Lrelu)
fused with the PSUM evacuation, and the big lc/l0/l1 weights are resident
in SBUF instead of streamed per-use.
"""
import sys

sys.path.insert(0, "/opt/trn_rl_repo")

import numpy as np

import concourse.bacc as bacc
import concourse.tile as tile
from concourse import mybir
from concourse.bass_utils import run_bass_kernel_spmd

F32 = mybir.dt.float32
F32R = mybir.dt.float32r
I16 = mybir.dt.int16
U16 = mybir.dt.uint16

B, N, K = 16, 1024, 20
N_CORES = 8
ELS = B // N_CORES
CH_C = [3, 64, 64, 128]
CH_O = [64, 64, 128, 256]
EMB = 1024
NT = N // 128
MMF = 512                     # fp32 matmul free-dim limit (one PSUM bank)
NEG = -1.0e30

AF = mybir.ActivationFunctionType
ALU = mybir.AluOpType
AX = mybir.AxisListType


def _mm(nc, out, lhsT, rhs, start, stop, rdt=True):
    if rdt:
        lhsT = lhsT.bitcast(F32R)
        rhs = rhs.bitcast(F32R)
    fd = rhs.shape[-1]
    if fd <= MMF:
        nc.tensor.matmul(out=out, lhsT=lhsT, rhs=rhs, start=start, stop=stop)
        return
    for f0 in range(0, fd, MMF):
        f1 = min(f0 + MMF, fd)
        nc.tensor.matmul(out=out[:, f0:f1], lhsT=lhsT, rhs=rhs[:, f0:f1],
                         start=start, stop=stop)


def _pool_kmax(nc, m_seg, g):
    """Neighbor max over K=20 on the Pool engine: in-place max tree inside
    the gathered tile g [128, 128*K], result -> m_seg [128, 128]."""
    gv = g.rearrange("p (i r) -> p i r", r=K)
    TT = nc.gpsimd.tensor_tensor
    TT(out=gv[:, :, 0:10], in0=gv[:, :, 0:10], in1=gv[:, :, 10:20], op=ALU.max)
    TT(out=gv[:, :, 0:5], in0=gv[:, :, 0:5], in1=gv[:, :, 5:10], op=ALU.max)
    TT(out=gv[:, :, 0:2], in0=gv[:, :, 0:2], in1=gv[:, :, 2:4], op=ALU.max)
    TT(out=gv[:, :, 0:1], in0=gv[:, :, 0:1], in1=gv[:, :, 1:2], op=ALU.max)
    TT(out=m_seg.rearrange("p (i r) -> p i r", r=1),
       in0=gv[:, :, 0:1], in1=gv[:, :, 4:5], op=ALU.max)


def build_program(debug=False, reps=1, ablate=()):
    nc = bacc.Bacc("TRN2", target_bir_lowering=False, debug=False)

    x_in = nc.dram_tensor("x3", [ELS * 3, N], F32, kind="ExternalInput")
    wa_d, wc_d, wt_d = [], [], []
    for l in range(4):
        C, O = CH_C[l], CH_O[l]
        wa_d.append(nc.dram_tensor(f"wa{l}", [C, O], F32, kind="ExternalInput"))
        wc_d.append(nc.dram_tensor(f"wc{l}", [C, O], F32, kind="ExternalInput"))
        wt_d.append(nc.dram_tensor(f"wt{l}", [1, O], F32, kind="ExternalInput"))
    wlc_d = nc.dram_tensor("wlc", [512, EMB], F32, kind="ExternalInput")
    lct_d = nc.dram_tensor("lct", [128, NT], F32, kind="ExternalInput")
    wl0_d = nc.dram_tensor("wl0", [2049, 512], F32, kind="ExternalInput")
    wl1_d = nc.dram_tensor("wl1", [513, 256], F32, kind="ExternalInput")
    wow_d = nc.dram_tensor("wow", [257, 40], F32, kind="ExternalInput")
    out_d = nc.dram_tensor("out", [ELS, 40], F32, kind="ExternalOutput")

    lc_rows = [(0, 64), (64, 128), (128, 256), (256, 384), (384, 512)]
    l0_rows = [(kc * 128, (kc + 1) * 128) for kc in range(16)] + [(2048, 2049)]
    l1_rows = [(kc * 128, (kc + 1) * 128) for kc in range(4)] + [(512, 513)]
    ow_rows = [(0, 128), (128, 256), (256, 257)]

    with tile.TileContext(nc) as tc:
        with (
            tc.tile_pool(name="w", bufs=1) as wpool,
            tc.tile_pool(name="y", bufs=1) as ypool,
            tc.tile_pool(name="s1", bufs=1) as spool1,
            tc.tile_pool(name="s", bufs=2) as spool,
            tc.tile_pool(name="pdp", bufs=3) as pdpool,
            tc.tile_pool(name="g", bufs=3) as gpool,
            tc.tile_pool(name="iwp", bufs=1) as iwpool,
            tc.tile_pool(name="dr", bufs=2, space="DRAM") as dramp,
            tc.tile_pool(name="jit", bufs=2) as jitp,
        ):
            # ---------------- consts + resident weights ----------------
            ones_row = wpool.tile([1, N], F32, tag="ones_row")
            ones_col = wpool.tile([128, 1], F32, tag="ones_col")
            ones2 = wpool.tile([1, ELS], F32, tag="ones2")
            eye2 = wpool.tile([ELS, ELS], F32, tag="eye2")
            nc.vector.memset(ones_row[:], 1.0)
            nc.vector.memset(ones_col[:], 1.0)
            nc.vector.memset(ones2[:], 1.0)
            nc.vector.memset(eye2[:], 0.0)
            for _e in range(ELS):
                nc.vector.memset(eye2[_e:_e + 1, _e:_e + 1], 1.0)

            x0_tiles = []
            for el in range(ELS):
                t = ypool.tile([3, N], F32, tag=f"x0_{el}", name=f"x0_{el}")
                nc.sync.dma_start(t[:], x_in.ap()[el * 3:(el + 1) * 3, :])
                x0_tiles.append(t)

            wa, wc, wt = [], [], []
            for l in range(4):
                C, O = CH_C[l], CH_O[l]
                ta = wpool.tile([C, O], F32, tag=f"wa{l}")
                tcc = wpool.tile([C, O], F32, tag=f"wc{l}")
                tt = wpool.tile([1, O], F32, tag=f"wt{l}")
                nc.sync.dma_start(ta[:], wa_d[l].ap())
                nc.sync.dma_start(tcc[:], wc_d[l].ap())
                nc.sync.dma_start(tt[:], wt_d[l].ap())
                wa.append(ta); wc.append(tcc); wt.append(tt)

            # resident lc weights + per-channel bias column, loaded once
            wlc_t = []
            for r0, r1 in lc_rows:
                t = wpool.tile([r1 - r0, EMB], F32, tag=f"wlc{r0}")
                nc.sync.dma_start(t[:], wlc_d.ap()[r0:r1, :])
                wlc_t.append(t)
            lct = wpool.tile([128, NT], F32, tag="lct")
            nc.sync.dma_start(lct[:], lct_d.ap())


            # h_parts[l][el] = list of ([<=128, N] AP) feature chunks (lc concat order)
            h_parts = [[None] * ELS for _ in range(4)]
            maxes = ypool.tile([128, NT, ELS], F32, tag="maxes")
            sums = ypool.tile([128, NT, ELS], F32, tag="sums")
            maxes2 = ypool.tile([128, NT, ELS, 2], F32, tag="maxes2")
            sums2 = ypool.tile([128, NT, ELS, 2], F32, tag="sums2")
            aug_t, raug_t = [], []
            for el in range(ELS):
                a1 = wpool.tile([2, N], F32, tag=f"aug{el}", name=f"aug{el}")
                a2 = wpool.tile([2, N], F32, tag=f"raug{el}", name=f"raug{el}")
                nc.sync.dma_start(a1[1:2, :], ones_row[:])
                nc.sync.dma_start(a2[0:1, :], ones_row[:])
                aug_t.append(a1); raug_t.append(a2)

            for _rep in range(reps):
              Xf = [x0_tiles[el][:] for el in range(ELS)]
              with (
                tc.tile_pool(name=f"pspd{_rep}", bufs=2, space="PSUM") as pspd,
                tc.tile_pool(name=f"psmm{_rep}", bufs=1, space="PSUM") as psmm,
                tc.tile_pool(name=f"pslc{_rep}", bufs=2, space="PSUM") as pslc,
              ):
                # ================= EdgeConv layers =================
                for l in range(4):
                    C, O = CH_C[l], CH_O[l]
                    packed = (O == 64 and ELS == 2)
                    nch = 1 if packed else O // 128  # gather-channel chunks per el

                    # per-tile wrapped+replicated idx tiles (separate tiles so a
                    # gather depends only on its own tile's topk, not all of them)
                    iw_t = {}
                    for t in range(NT):
                        if packed:
                            w = iwpool.tile([128, 160], I16, tag=f"iw{t}", name=f"iw{t}")
                            for el in range(ELS):
                                iw_t[(el, t)] = w
                        else:
                            for el in range(ELS):
                                w = iwpool.tile([128, 160], I16, tag=f"iw{el}_{t}",
                                                name=f"iw{el}_{t}")
                                iw_t[(el, t)] = w

                    # per-el prep: xx row + aug rows (pd/2 variant: the 2x
                    # rhs scaling is dropped; ordering is unchanged)
                    flats = []
                    for el in range(ELS):
                        xf = Xf[el]
                        xsq = spool1.tile([C, N], F32, tag=f"xsq{el}")
                        nc.scalar.activation(out=xsq[:], in_=xf, func=AF.Square)
                        xx_ps = psmm.tile([1, N], F32, tag="mm")
                        _mm(nc, xx_ps[:], ones_col[0:C, :], xsq[:], True, True)
                        # pd/2 = G + nxx_i/2 + nxx_j/2: aug = [nxx/2; 1]
                        aug, raug = aug_t[el], raug_t[el]
                        nc.scalar.activation(out=aug[0:1, :], in_=xx_ps[:], func=AF.Copy,
                                             scale=-0.5)
                        nc.sync.dma_start(raug[1:2, :], aug[0:1, :])
                        flat = dramp.tile([NT * 128, K], I16, tag=f"idxflat{el}",
                                          name=f"idxflat{el}")
                        flats.append(flat)

                    # topk per tile
                    for el in range(ELS):
                        for t in range(NT):
                            xf = Xf[el]
                            iw = iw_t[(el, t)]
                            p_base = 64 * el if packed else 0
                            nrep = 4 if packed else 8
                            aug, raug = aug_t[el], raug_t[el]
                            flat = flats[el]

                            pd_ps = pspd.tile([128, N], F32, tag="pd")
                            _mm(nc, pd_ps[:], xf[:, t * 128:(t + 1) * 128], xf,
                                True, False)
                            _mm(nc, pd_ps[:], aug[:, t * 128:(t + 1) * 128], raug[:],
                                False, True)
                            pd_sb = pdpool.tile([128, N], F32, tag="pdsb")
                            nc.scalar.activation(out=pd_sb[:], in_=pd_ps[:], func=AF.Copy)
                            v = pdpool.tile([128, 24], F32, tag="v")
                            vi = pdpool.tile([128, 24], U16, tag="vi")
                            nc.vector.max(out=v[:, 0:8], in_=pd_sb[:])
                            nc.vector.max_index(out=vi[:, 0:8], in_max=v[:, 0:8],
                                                in_values=pd_sb[:])
                            nc.vector.match_replace(out=pd_sb[:], in_to_replace=v[:, 0:8],
                                                    in_values=pd_sb[:], imm_value=NEG)
                            nc.vector.max(out=v[:, 8:16], in_=pd_sb[:])
                            nc.vector.max_index(out=vi[:, 8:16], in_max=v[:, 8:16],
                                                in_values=pd_sb[:])
                            nc.vector.match_replace(out=pd_sb[:], in_to_replace=v[:, 8:16],
                                                    in_values=pd_sb[:], imm_value=NEG)
                            nc.vector.max(out=v[:, 16:24], in_=pd_sb[:])
                            nc.vector.max_index(out=vi[:, 16:24], in_max=v[:, 16:24],
                                                in_values=pd_sb[:])
                            # e-order dump -> DRAM, then wrapped([16,160]) via
                            # per-tile: e-order dump -> wrapped strided read
                            # -> log2 replication (pipelines under rounds)
                            nc.sync.dma_start(flat[t * 128:(t + 1) * 128, :],
                                              vi[:, 0:K].bitcast(I16))
                            src = (flat[t * 128:(t + 1) * 128, :]
                                   .rearrange("p r -> (p r)")
                                   .rearrange("(s w) -> w s", w=16))
                            nc.sync.dma_start(iw[p_base:p_base + 16, :], src)
                            blk = 16
                            while blk < 16 * nrep:
                                nc.sync.dma_start(
                                    iw[p_base + blk:p_base + 2 * blk, :],
                                    iw[p_base:p_base + blk, :])
                                blk *= 2

                    # ---- convs + gather + activation ----
                    if packed:
                        a_sb = spool.tile([128, N], F32, tag="asb")
                        c_sb = spool.tile([128, N], F32, tag="csb")
                        for el in range(ELS):
                            a_ps = psmm.tile([64, N], F32, tag="mm")
                            _mm(nc, a_ps[:], wa[l][:, 0:O], Xf[el], True, True)
                            nc.scalar.activation(out=a_sb[64 * el:64 * (el + 1), :],
                                                 in_=a_ps[:], func=AF.Copy)
                            c_ps = psmm.tile([64, N], F32, tag="mm")
                            _mm(nc, c_ps[:], wc[l][:, 0:O], Xf[el], True, False)
                            _mm(nc, c_ps[:], wt[l][:, 0:O], ones_row[:], False, True)
                            nc.scalar.activation(out=c_sb[64 * el:64 * (el + 1), :],
                                                 in_=c_ps[:], func=AF.Copy)
                        m_sb = spool.tile([128, N], F32, tag="msb")
                        for t in range(NT):
                            g = gpool.tile([128, 2560], F32, tag="gath")
                            nc.gpsimd.ap_gather(
                                out_ap=g[:], in_ap=a_sb[:],
                                idxs_ap=iw_t[(0, t)][:],
                                channels=128, num_elems=N, d=1, num_idxs=2560)
                            _pool_kmax(nc, m_sb[:, t * 128:(t + 1) * 128], g[:])
                        u = spool.tile([128, N], F32, tag="u")
                        nc.gpsimd.tensor_tensor(out=u[:], in0=m_sb[:], in1=c_sb[:],
                                                op=ALU.add)
                        newX = []
                        for el in range(ELS):
                            yt = ypool.tile([64, N], F32, tag=f"y{l}_{el}",
                                            name=f"y{l}_{el}")
                            nc.scalar.activation(out=yt[:],
                                                 in_=u[64 * el:64 * (el + 1), :],
                                                 func=AF.Lrelu, alpha=0.2)
                            h_parts[l][el] = [yt[:]]
                            newX.append(yt[:])
                        Xf = newX
                    else:
                        newX = [None] * ELS
                        for el in range(ELS):
                            ychunks = []
                            for ch in range(nch):
                                o0, o1 = ch * 128, (ch + 1) * 128
                                a_sb = spool.tile([128, N], F32, tag="asb")
                                a_ps = psmm.tile([128, N], F32, tag="mm")
                                _mm(nc, a_ps[:], wa[l][:, o0:o1], Xf[el], True, True)
                                nc.scalar.activation(out=a_sb[:], in_=a_ps[:],
                                                     func=AF.Copy)
                                c_ps = psmm.tile([128, N], F32, tag="mm")
                                _mm(nc, c_ps[:], wc[l][:, o0:o1], Xf[el], True, False)
                                _mm(nc, c_ps[:], wt[l][:, o0:o1], ones_row[:],
                                    False, True)
                                c_sb = spool.tile([128, N], F32, tag="csb")
                                nc.scalar.activation(out=c_sb[:], in_=c_ps[:],
                                                     func=AF.Copy)
                                m_sb = spool.tile([128, N], F32, tag="msb")
                                for t in range(NT):
                                    g = gpool.tile([128, 2560], F32, tag="gath")
                                    nc.gpsimd.ap_gather(
                                        out_ap=g[:], in_ap=a_sb[:],
                                        idxs_ap=iw_t[(el, t)][:],
                                        channels=128, num_elems=N, d=1, num_idxs=2560)
                                    if l == 3:
                                        nc.vector.tensor_reduce(
                                            out=m_sb[:, t * 128:(t + 1) * 128],
                                            in_=g[:].rearrange("p (i r) -> p i r", r=K),
                                            axis=AX.X, op=ALU.max)
                                    else:
                                        _pool_kmax(nc, m_sb[:, t * 128:(t + 1) * 128],
                                                   g[:])
                                u = spool.tile([128, N], F32, tag="u")
                                nc.gpsimd.tensor_tensor(out=u[:], in0=m_sb[:],
                                                        in1=c_sb[:], op=ALU.add)
                                yt = ypool.tile([128, N], F32, tag=f"y{l}_{el}_{ch}")
                                nc.scalar.activation(out=yt[:], in_=u[:],
                                                     func=AF.Lrelu, alpha=0.2)
                                ychunks.append(yt[:])
                            h_parts[l][el] = ychunks
                            if nch == 1:
                                newX[el] = ychunks[0]
                        if l < 3:
                            Xf = newX

                    # ================= lc conv + pooling =================
                for el in range(ELS):
                    rhs_chunks = (h_parts[0][el] + h_parts[1][el] + h_parts[2][el]
                                  + h_parts[3][el])
                    for mt in range(8):
                      for h in range(2):
                        f0, f1 = h * 512, (h + 1) * 512
                        u_ps = pslc.tile([128, 512], F32, tag="lc")
                        for kc in range(5):
                            _mm(nc, u_ps[:], wlc_t[kc][:, mt * 128:(mt + 1) * 128],
                                rhs_chunks[kc][:, f0:f1], kc == 0, kc == 4)
                        y5 = spool.tile([128, 512], F32, tag="y5")
                        nc.scalar.activation(
                            out=y5[:], in_=u_ps[:], func=AF.Lrelu,
                            alpha=0.2, bias=lct[:, mt:mt + 1],
                            accum_out=sums2[:, mt:mt + 1, el:el + 1, h:h + 1])
                        nc.vector.tensor_reduce(
                            out=maxes2[:, mt:mt + 1, el:el + 1, h:h + 1],
                            in_=y5[:], axis=AX.X, op=ALU.max)
                # combine the two half-N partials
                nc.vector.tensor_tensor(
                    out=maxes[:], in0=maxes2[:, :, :, 0], in1=maxes2[:, :, :, 1],
                    op=ALU.max)
                nc.vector.tensor_tensor(
                    out=sums[:], in0=sums2[:, :, :, 0], in1=sums2[:, :, :, 1],
                    op=ALU.add)

              # ====== FC head: batch (ELS) on PSUM partitions, wide free dims ======
              with tc.tile_pool(name=f"psfc{_rep}", bufs=1, space="PSUM") as psfc:
                  # l0: out [ELS, 512] accumulated over 17 contract chunks
                  l0ps = psfc.tile([ELS, 512], F32, tag="fc0")
                  for kc in range(17):
                      r0, r1 = l0_rows[kc]
                      wj = jitp.tile([r1 - r0, 512], F32, tag="wj0")
                      nc.sync.dma_start(wj[:], wl0_d.ap()[r0:r1, :])
                      if kc < 8:
                          lhsT = maxes[:, kc:kc + 1, :].rearrange("p a b -> p (a b)")
                      elif kc < 16:
                          lhsT = sums[:, kc - 8:kc - 7, :].rearrange("p a b -> p (a b)")
                      else:
                          lhsT = ones2[:]
                      nc.tensor.matmul(out=l0ps[:], lhsT=lhsT, rhs=wj[:],
                                       start=kc == 0, stop=kc == 16)
                  y6 = spool.tile([ELS, 512], F32, tag="y6")
                  nc.scalar.activation(out=y6[:], in_=l0ps[:], func=AF.Lrelu,
                                       alpha=0.2)
                  # transpose y6 -> [128, 4, ELS] via 4 PE transposes
                  y6tp = psfc.tile([128, 4, ELS], F32, tag="fct")
                  for c in range(4):
                      nc.tensor.transpose(out=y6tp[:, c, :],
                                          in_=y6[:, c * 128:(c + 1) * 128],
                                          identity=eye2[:])
                  y6t = spool.tile([128, 4 * ELS], F32, tag="y6t")
                  y6tv = y6t[:].rearrange("p (a b) -> p a b", a=4)
                  nc.scalar.activation(out=y6t[:],
                                       in_=y6tp[:].rearrange("p a b -> p (a b)"),
                                       func=AF.Copy)
                  # l1: out [ELS, 256] over 4 contract chunks + bias row
                  l1ps = psfc.tile([ELS, 256], F32, tag="fc1")
                  for kc in range(5):
                      r0, r1 = l1_rows[kc]
                      wj = jitp.tile([r1 - r0, 256], F32, tag="wj1")
                      nc.sync.dma_start(wj[:], wl1_d.ap()[r0:r1, :])
                      lhsT = (y6tv[:, kc, :] if kc < 4 else ones2[:])
                      nc.tensor.matmul(out=l1ps[:], lhsT=lhsT, rhs=wj[:],
                                       start=kc == 0, stop=kc == 4)
                  y7 = spool.tile([ELS, 256], F32, tag="y7")
                  nc.scalar.activation(out=y7[:], in_=l1ps[:], func=AF.Lrelu,
                                       alpha=0.2)
                  y7tp = psfc.tile([128, 2, ELS], F32, tag="fct2")
                  for c in range(2):
                      nc.tensor.transpose(out=y7tp[:, c, :],
                                          in_=y7[:, c * 128:(c + 1) * 128],
                                          identity=eye2[:])
                  y7t = spool.tile([128, 2 * ELS], F32, tag="y7t")
                  y7tv = y7t[:].rearrange("p (a b) -> p a b", a=2)
                  nc.scalar.activation(out=y7t[:],
                                       in_=y7tp[:].rearrange("p a b -> p (a b)"),
                                       func=AF.Copy)
                  ops_ = psfc.tile([ELS, 40], F32, tag="fcout")
                  for kc in range(3):
                      r0, r1 = ow_rows[kc]
                      wj = jitp.tile([r1 - r0, 40], F32, tag=f"wjo{kc}")
                      nc.sync.dma_start(wj[:], wow_d.ap()[r0:r1, :])
                      lhsT = (y7tv[:, kc, :] if kc < 2 else ones2[:])
                      nc.tensor.matmul(out=ops_[:], lhsT=lhsT, rhs=wj[:],
                                       start=kc == 0, stop=kc == 2)
                  osb = spool.tile([ELS, 40], F32, tag="osb")
                  nc.scalar.activation(out=osb[:], in_=ops_[:], func=AF.Copy)
                  nc.sync.dma_start(out_d.ap(), osb[:])

    nc.compile()
    return nc


def _fold_weights(i):
    out = {}
    for l in range(4):
        C = CH_C[l]
        w = np.asarray(i[f"c{l}_w"], np.float64)
        b = np.asarray(i[f"c{l}_b"], np.float64)
        g = np.asarray(i[f"c{l}_g"], np.float64)
        be = np.asarray(i[f"c{l}_be"], np.float64)
        m = np.asarray(i[f"c{l}_m"], np.float64)
        v = np.asarray(i[f"c{l}_v"], np.float64)
        s = g / np.sqrt(v + 1e-5)
        w1, w2 = w[:, :C], w[:, C:]
        out[f"wa{l}"] = np.ascontiguousarray((s[:, None] * w1).T).astype(np.float32)
        out[f"wc{l}"] = np.ascontiguousarray((s[:, None] * (w2 - w1)).T).astype(np.float32)
        out[f"wt{l}"] = (s * b + be - s * m).astype(np.float32)[None, :]
    s = np.asarray(i["lc_g"], np.float64) / np.sqrt(np.asarray(i["lc_v"], np.float64) + 1e-5)
    t = s * np.asarray(i["lc_b"], np.float64) + np.asarray(i["lc_be"], np.float64) \
        - s * np.asarray(i["lc_m"], np.float64)
    out["wlc"] = np.ascontiguousarray(
        np.concatenate([(s[:, None] * np.asarray(i["lc_w"], np.float64)).T,
                        t[None, :]], 0)).astype(np.float32)
    s = np.asarray(i["l0_g"], np.float64) / np.sqrt(np.asarray(i["l0_v"], np.float64) + 1e-5)
    t = np.asarray(i["l0_be"], np.float64) - s * np.asarray(i["l0_m"], np.float64)
    w = s[:, None] * np.asarray(i["l0_w"], np.float64)
    w[:, 1024:] /= 1024.0
    out["wl0"] = np.ascontiguousarray(np.concatenate([w.T, t[None, :]], 0)).astype(np.float32)
    s = np.asarray(i["l1_g"], np.float64) / np.sqrt(np.asarray(i["l1_v"], np.float64) + 1e-5)
    t = s * np.asarray(i["l1_b"], np.float64) + np.asarray(i["l1_be"], np.float64) \
        - s * np.asarray(i["l1_m"], np.float64)
    out["wl1"] = np.ascontiguousarray(
        np.concatenate([(s[:, None] * np.asarray(i["l1_w"], np.float64)).T,
                        t[None, :]], 0)).astype(np.float32)
    out["wow"] = np.ascontiguousarray(
        np.concatenate([np.asarray(i["ow"], np.float32).T,
                        np.asarray(i["ob"], np.float32)[None, :]], 0))
    return out


_NC_CACHE = {}


def get_program(debug=False):
    if debug not in _NC_CACHE:
        _NC_CACHE[debug] = build_program(debug)
    return _NC_CACHE[debug]


def make_in_maps(inputs):
    folded = _fold_weights(inputs)
    x = np.asarray(inputs["x"], np.float32)
    in_maps = []
    for c in range(N_CORES):
        m = dict(folded)
        xs = x[c * ELS:(c + 1) * ELS]                       # (ELS, 1024, 3)
        m["x3"] = np.ascontiguousarray(
            xs.transpose(0, 2, 1).reshape(ELS * 3, N))
        in_maps.append(m)
    return in_maps


def kernel(**inputs) -> np.ndarray:
    nc = get_program(False)
    in_maps = make_in_maps(inputs)
    res = run_bass_kernel_spmd(nc, in_maps, list(range(N_CORES)))
    outs = [res.results[c]["out"] for c in range(N_CORES)]
    return np.concatenate(outs, 0).astype(np.float32)
